# revision 17
# baseline (speedup 1.0000x reference)
"""Pairwise-distance adjacency kernel (exp(-||a-b||)) for Trainium2, 8 cores.

Problem: inputs1 [4,4096,256], inputs2 [4,4096,256] (fp32)
         out[b,n,m] = exp(-sqrt(clip(||a_bn||^2 - 2 a.b + ||b_bm||^2)))

Sharding: 8 shards = (batch b in 0..3) x (row-half h in 0..1) of inputs1.
Each core computes a [2048, 4096] block of the output for one batch.

Per-core pipeline:
  - host ships K-major aT [256,2048] and bT [256,4096] (layout prep only)
  - device computes row norms na, nb (square + ones-matmul reduction)
  - norms are folded into the matmul as an extra "augmented" K-subtile:
      psum = a.b - na/2 - nb/2   (3 fp32r matmuls per [128,512] chunk)
  - ScalarE pass 1: D = Sqrt(-2 * psum)   (PSUM -> SBUF staging)
  - ScalarE pass 2: out = Exp(-D)         (in place; table switches batched
    in groups of G row-tiles since sqrt/exp live in different ACT table sets)
  - DMA staging -> DRAM
"""

import os
import sys

for _p in ("/opt/trn_rl_repo", "/root/.axon_site/_ro/trn_rl_repo"):
    if os.path.isdir(_p) and _p not in sys.path:
        sys.path.append(_p)

import numpy as np

import concourse.bass as bass
import concourse.mybir as mybir
from concourse import bacc
from concourse.tile import TileContext
from concourse.bass_utils import run_bass_kernel_spmd

F32 = mybir.dt.float32
F32R = mybir.dt.float32r
AL = mybir.AluOpType
AF = mybir.ActivationFunctionType

P = 128          # partitions
D = 256          # feature dim (contraction)
KS = D // P      # 2 K-subtiles
M = 2048         # rows per core (inputs1 shard)
N = 4096         # cols per core (full inputs2 rows for one batch)
MT = M // P      # 16 m-tiles
NCH = 512        # matmul free-dim chunk (one PSUM bank)
PSW = 2048       # psum tile width (4 banks); 2 tiles = all 8 banks
G = 6            # row-tiles per ACT table-set group (staging bufs)

B_FULL, N_FULL = 4, 4096
N_CORES = 8

_nc_cache = None


def _build():
    """Build the single-core Bass program (identical on all 8 cores)."""
    nc = bacc.Bacc()
    # fp32r = fp32 bytes; declaring the DRAM side as fp32r lets the fast
    # HWDGE dma path load straight into fp32r SBUF tiles with no cast.
    aT_d = nc.declare_dram_parameter("aT", [D, M], F32R, isOutput=False)
    bT_d = nc.declare_dram_parameter("bT", [D, N], F32R, isOutput=False)
    out_d = nc.declare_dram_parameter("o", [M, N], F32, isOutput=True)
    dbg_d = nc.declare_dram_parameter("dbg", [P, N], F32, isOutput=True)

    out_r = out_d[:, :].rearrange("(t p) n -> t p n", p=P)

    with TileContext(nc) as tc:
        with (
            tc.tile_pool(name="const", bufs=1) as const,
            tc.tile_pool(name="psum", bufs=2, space="PSUM") as psum,
        ):
            # Augmented K-major operands: k-subtiles 0..1 hold the data,
            # k-subtile 2 holds the norm rows:
            #   aT_aug[0, 2, m] = na[m]   bT_aug[0, 2, n] = -0.5
            #   aT_aug[1, 2, m] = 1.0     bT_aug[1, 2, n] = -nb[n]/2
            # so the 3rd matmul adds -(na+nb)/2 into psum.
            # walrus requires every producer feeding an fp32r matmul to
            # emit fp32r, so the operand tiles are natively float32r and
            # the loads go through the casting DMA path (gpsimd).
            aT_aug = const.tile([P, KS + 1, M], F32R)
            bT_aug = const.tile([P, KS + 1, N], F32R)
            ones2 = const.tile([P, 2], F32)

            nc.sync.dma_start(
                out=aT_aug[:, 0:KS, :],
                in_=aT_d[:, :].rearrange("(ks p) m -> p ks m", p=P),
            )
            nc.sync.dma_start(
                out=bT_aug[:, 0:KS, :],
                in_=bT_d[:, :].rearrange("(ks p) n -> p ks n", p=P),
            )
            # DVE ops must start at partition 0: write rows [0:2] then
            # overwrite row 0, instead of writing row 1 directly.
            # Memset has no fp32r encoding -> write the bit patterns as u32.
            U32 = mybir.dt.uint32
            nc.vector.memset(aT_aug[:, KS, :].bitcast(U32), 0)
            nc.vector.memset(bT_aug[:, KS, :].bitcast(U32), 0)
            nc.vector.memset(aT_aug[0:2, KS, :].bitcast(U32),
                             0x3F800000)  # 1.0; row 0 replaced by na below
            nc.vector.memset(ones2[:, :], 1.0)

            # --- norms: na[m] = sum_k aT[k,m]^2 (row vector via ones-matmul,
            # duplicated on psum partitions 0 and 1 so each target partition
            # can be written by its own DVE lane) ---
            with tc.tile_pool(name="tmp", bufs=1) as tmp:
                asq = tmp.tile([P, KS, M], F32, tag="asq")
                nc.vector.tensor_tensor(
                    out=asq[:], in0=aT_aug[:, 0:KS, :], in1=aT_aug[:, 0:KS, :],
                    op=AL.mult,
                )
                pna = psum.tile([P, PSW], F32, tag="pt")
                for c in range(M // NCH):
                    for k in range(KS):
                        nc.tensor.matmul(
                            pna[0:2, c * NCH:(c + 1) * NCH],
                            lhsT=ones2[:, :],
                            rhs=asq[:, k, c * NCH:(c + 1) * NCH],
                            start=(k == 0),
                            stop=(k == KS - 1),
                        )
                # PSUM -> fp32r writes are rejected by walrus; bounce through
                # an f32 SBUF scratch. Per-bank ops keep sync waits low.
                bounce = tmp.tile([P, PSW], F32, tag="bounce")
                for c in range(M // NCH):
                    sl = slice(c * NCH, (c + 1) * NCH)
                    nc.vector.tensor_copy(out=bounce[0:1, sl], in_=pna[0:1, sl])
                    nc.vector.tensor_copy(
                        out=aT_aug[0:1, KS, sl], in_=bounce[0:1, sl],
                    )

                bsq = tmp.tile([P, KS, N], F32, tag="bsq")
                nc.vector.tensor_tensor(
                    out=bsq[:], in0=bT_aug[:, 0:KS, :], in1=bT_aug[:, 0:KS, :],
                    op=AL.mult,
                )
                for half in range(N // PSW):
                    pnb = psum.tile([P, PSW], F32, tag="pt")
                    for c in range(PSW // NCH):
                        n0 = half * PSW + c * NCH
                        for k in range(KS):
                            nc.tensor.matmul(
                                pnb[0:2, c * NCH:(c + 1) * NCH],
                                lhsT=ones2[:, :],
                                rhs=bsq[:, k, n0:n0 + NCH],
                                start=(k == 0),
                                stop=(k == KS - 1),
                            )
                    bounce2 = tmp.tile([P, PSW], F32, tag="bounce")
                    for c in range(PSW // NCH):
                        sl = slice(c * NCH, (c + 1) * NCH)
                        nc.vector.tensor_copy(out=bounce2[0:2, sl],
                                              in_=pnb[0:2, sl])
                        nc.vector.tensor_scalar_mul(
                            bT_aug[0:2, KS,
                                   half * PSW + c * NCH:half * PSW + (c + 1) * NCH],
                            bounce2[0:2, sl],
                            -0.5,
                        )
                # row 0 of the aug subtile must be the constant -0.5
                # (row 1 keeps -nb/2 from the write above)
                nc.vector.memset(bT_aug[0:1, KS, :].bitcast(U32), 0xBF000000)

            # --- main loop ---
            with tc.tile_pool(name="stage", bufs=G) as stage_pool:
                staged = []

                def flush():
                    for st_, i_ in staged:
                        nc.scalar.activation(
                            out=st_[:], in_=st_[:], func=AF.Exp,
                            bias=0.0, scale=-1.0,
                        )
                        nc.sync.dma_start(out=out_r[i_], in_=st_[:])
                    staged.clear()

                for i in range(MT):
                    st = stage_pool.tile([P, N], F32, tag="stage")
                    m0 = i * P
                    for half in range(N // PSW):
                        pt = psum.tile([P, PSW], F32, tag="pt")
                        for c in range(PSW // NCH):
                            n0 = half * PSW + c * NCH
                            ps = pt[:, c * NCH:(c + 1) * NCH]
                            for k in range(KS + 1):
                                nc.tensor.matmul(
                                    ps,
                                    lhsT=aT_aug[:, k, m0:m0 + P],
                                    rhs=bT_aug[:, k, n0:n0 + NCH],
                                    start=(k == 0),
                                    stop=(k == KS),
                                )
                        nc.scalar.activation(
                            out=st[:, half * PSW:(half + 1) * PSW],
                            in_=pt[:],
                            func=AF.Sqrt,
                            bias=0.0,
                            scale=-2.0,
                        )
                    if i == 0:
                        # debug tap: D values of the first row-tile
                        nc.sync.dma_start(out=dbg_d[:, :], in_=st[:])
                    staged.append((st, i))
                    if len(staged) == G or i == MT - 1:
                        flush()

    nc.compile()
    return nc


def _get_nc():
    global _nc_cache
    if _nc_cache is None:
        _nc_cache = _build()
    return _nc_cache


def _make_in_maps(inputs1, inputs2):
    inputs1 = np.asarray(inputs1, dtype=np.float32)
    inputs2 = np.asarray(inputs2, dtype=np.float32)
    in_maps = []
    for c in range(N_CORES):
        b, h = divmod(c, 2)
        in_maps.append({
            "aT": np.ascontiguousarray(inputs1[b, h * M:(h + 1) * M, :].T),
            "bT": np.ascontiguousarray(inputs2[b].T),
        })
    return in_maps


def _run_spmd(inputs1, inputs2, trace=False):
    nc = _get_nc()
    in_maps = _make_in_maps(inputs1, inputs2)
    return run_bass_kernel_spmd(nc, in_maps, core_ids=list(range(N_CORES)),
                                trace=trace)


def _assemble(results):
    out = np.empty((B_FULL, 2 * M, N_FULL), np.float32)
    for c in range(N_CORES):
        b, h = divmod(c, 2)
        out[b, h * M:(h + 1) * M, :] = results[c]["o"]
    return out


def kernel(inputs1, inputs2):
    res = _run_spmd(inputs1, inputs2, trace=False)
    return _assemble(res.results)


# revision 18
# speedup vs baseline: 1.1411x; 1.1411x over previous
"""Pairwise-distance adjacency kernel (exp(-||a-b||)) for Trainium2, 8 cores.

Problem: inputs1 [4,4096,256], inputs2 [4,4096,256] (fp32)
         out[b,n,m] = exp(-sqrt(clip(||a_bn||^2 - 2 a.b + ||b_bm||^2)))

Sharding: 8 shards = (batch b in 0..3) x (row-half h in 0..1) of inputs1.
Each core computes a [2048, 4096] block of the output for one batch.

Per-core pipeline (v2):
  - host ships K-major aT [256,2048], bT [256,4096] (fp32r matmul operands)
    and row-major a [2048,256] (for the na reduction layout)
  - na[m] per-partition via ScalarE Square+accum_out -> ACT1 bias (fp32)
  - nb[n] via square + f32r ones-matmul -> row vector; /2; replicated to
    all partitions with a K=1 outer-product matmul -> nbh_repl (fp32)
  - main loop per [128,2048] psum tile: 8 fp32r matmuls (k-subtiles 0,1)
    -> psum = a.b
  - VectorE: u = psum - nbh_repl          (PSUM -> SBUF staging)
  - ScalarE pass 1: D = Sqrt(-2*u + na)   (in place, bias=na per-partition)
  - ScalarE pass 2: out = Exp(-D)         (in place; sqrt/exp table switches
    batched in groups of G row-tiles, enforced with explicit deps)
  - DMA staging -> DRAM
"""

import os
import sys

for _p in ("/opt/trn_rl_repo", "/root/.axon_site/_ro/trn_rl_repo"):
    if os.path.isdir(_p) and _p not in sys.path:
        sys.path.append(_p)

import numpy as np

import concourse.bass as bass
import concourse.mybir as mybir
from concourse import bacc
from concourse.tile import TileContext, add_dep_helper
from concourse.bass_utils import run_bass_kernel_spmd

F32 = mybir.dt.float32
F32R = mybir.dt.float32r
U32 = mybir.dt.uint32
AL = mybir.AluOpType
AF = mybir.ActivationFunctionType

P = 128          # partitions
D = 256          # feature dim (contraction)
KS = D // P      # 2 K-subtiles
M = 2048         # rows per core (inputs1 shard)
N = 4096         # cols per core (full inputs2 rows for one batch)
MT = M // P      # 16 m-tiles
NCH = 512        # matmul free-dim chunk (one PSUM bank)
PSW = 2048       # psum tile width (4 banks); 2 tiles = all 8 banks
G = 6            # row-tiles per ACT table-set group (staging bufs)

B_FULL, N_FULL = 4, 4096
N_CORES = 8

_nc_cache = None


def _raw(inst):
    return getattr(inst, "ins", inst)


def _build():
    """Build the single-core Bass program (identical on all 8 cores)."""
    nc = bacc.Bacc()
    aT_d = nc.declare_dram_parameter("aT", [D, M], F32R, isOutput=False)
    bT_d = nc.declare_dram_parameter("bT", [D, N], F32R, isOutput=False)
    a_d = nc.declare_dram_parameter("a", [M, D], F32, isOutput=False)
    out_d = nc.declare_dram_parameter("o", [M, N], F32, isOutput=True)
    dbg_d = nc.declare_dram_parameter("dbg", [P, N], F32, isOutput=True)

    out_r = out_d[:, :].rearrange("(t p) n -> t p n", p=P)

    with TileContext(nc) as tc:
        with (
            tc.tile_pool(name="const", bufs=1) as const,
            tc.tile_pool(name="psum", bufs=2, space="PSUM") as psum,
        ):
            aT_r = const.tile([P, KS, M], F32R)
            bT_r = const.tile([P, KS, N], F32R)
            a_rm = const.tile([P, MT, D], F32)
            na_pm = const.tile([P, MT], F32)      # per-partition na bias
            nbh_repl = const.tile([P, N], F32)    # nb/2 on every partition
            ones2 = const.tile([P, 2], F32R)      # norm-reduce lhsT
            ones_row = const.tile([P, P], F32R)   # outer-product lhsT [1,128]

            nc.sync.dma_start(
                out=aT_r[:], in_=aT_d[:, :].rearrange("(ks p) m -> p ks m", p=P))
            nc.sync.dma_start(
                out=bT_r[:], in_=bT_d[:, :].rearrange("(ks p) n -> p ks n", p=P))
            nc.sync.dma_start(
                out=a_rm[:], in_=a_d[:, :].rearrange("(t p) k -> p t k", p=P))
            nc.vector.memset(ones2[:, :].bitcast(U32), 0x3F800000)
            nc.vector.memset(ones_row[0:1, :].bitcast(U32), 0x3F800000)

            with tc.tile_pool(name="tmp", bufs=1) as tmp:
                # --- na (per-partition, fp32 exact): Square + free-dim accum
                sq_scr = tmp.tile([P, D], F32, tag="sqscr")
                for t in range(MT):
                    nc.scalar.activation(
                        out=sq_scr[:], in_=a_rm[:, t, :], func=AF.Square,
                        accum_out=na_pm[:, t:t + 1],
                    )

                # --- nb (free-major row vector) via f32r ones-matmul
                bsq = tmp.tile([P, KS, N], F32R, tag="bsq")
                nc.vector.tensor_tensor(
                    out=bsq[:], in0=bT_r[:], in1=bT_r[:], op=AL.mult)
                nbh_row = tmp.tile([P, N], F32, tag="nbrow")
                nbh_row_r = tmp.tile([P, N], F32R, tag="nbrowr")
                for half in range(N // PSW):
                    pnb = psum.tile([P, PSW], F32, tag="pt")
                    for c in range(PSW // NCH):
                        n0 = half * PSW + c * NCH
                        for k in range(KS):
                            nc.tensor.matmul(
                                pnb[0:2, c * NCH:(c + 1) * NCH],
                                lhsT=ones2[:, :],
                                rhs=bsq[:, k, n0:n0 + NCH],
                                start=(k == 0),
                                stop=(k == KS - 1),
                            )
                    for c in range(PSW // NCH):
                        sl_l = slice(c * NCH, (c + 1) * NCH)
                        sl_g = slice(half * PSW + c * NCH,
                                     half * PSW + (c + 1) * NCH)
                        # nb/2 as fp32 row (DVE lane 0)
                        nc.vector.tensor_scalar_mul(
                            nbh_row[0:1, sl_g], pnb[0:1, sl_l], 0.5)
                        nc.vector.tensor_copy(
                            out=nbh_row_r[0:1, sl_g], in_=nbh_row[0:1, sl_g])

                # replicate nb/2 across partitions: ones[1,128] x row[1,nch]
                for c in range(N // NCH):
                    prep = psum.tile([P, PSW], F32, tag="pt")
                    sl = slice(c * NCH, (c + 1) * NCH)
                    nc.tensor.matmul(
                        prep[:, 0:NCH],
                        lhsT=ones_row[0:1, :],
                        rhs=nbh_row_r[0:1, sl],
                        start=True, stop=True,
                    )
                    nc.vector.tensor_copy(out=nbh_repl[:, sl],
                                          in_=prep[:, 0:NCH])

            # --- main loop ---
            with tc.tile_pool(name="stage", bufs=G) as stage_pool:
                staged = []
                last_exp = [None]
                first_sqrt_after_flush = [None]

                def flush():
                    first_exp = None
                    for st_, i_, last_sqrt in staged:
                        e = nc.scalar.activation(
                            out=st_[:], in_=st_[:], func=AF.Exp,
                            bias=0.0, scale=-1.0,
                        )
                        if first_exp is None:
                            first_exp = e
                            # no exp before the group's last sqrt
                            add_dep_helper(_raw(e), _raw(last_sqrt),
                                           reason="act-table group: exp after sqrt")
                        last_exp[0] = e
                        nc.sync.dma_start(out=out_r[i_], in_=st_[:])
                    staged.clear()

                for i in range(MT):
                    st = stage_pool.tile([P, N], F32, tag="stage")
                    m0 = i * P
                    last_sqrt = None
                    for half in range(N // PSW):
                        hsl = slice(half * PSW, (half + 1) * PSW)
                        pt = psum.tile([P, PSW], F32, tag="pt")
                        for c in range(PSW // NCH):
                            n0 = half * PSW + c * NCH
                            ps = pt[:, c * NCH:(c + 1) * NCH]
                            for k in range(KS):
                                nc.tensor.matmul(
                                    ps,
                                    lhsT=aT_r[:, k, m0:m0 + P],
                                    rhs=bT_r[:, k, n0:n0 + NCH],
                                    start=(k == 0),
                                    stop=(k == KS - 1),
                                )
                        # u = a.b - nb/2  (PSUM -> SBUF staging)
                        nc.vector.tensor_tensor(
                            out=st[:, hsl], in0=pt[:], in1=nbh_repl[:, hsl],
                            op=AL.subtract,
                        )
                        # D = sqrt(-2*u + na)
                        s = nc.scalar.activation(
                            out=st[:, hsl], in_=st[:, hsl], func=AF.Sqrt,
                            bias=na_pm[:, i:i + 1], scale=-2.0,
                        )
                        if last_exp[0] is not None and half == 0:
                            # no sqrt of this group before last group's exps
                            add_dep_helper(_raw(s), _raw(last_exp[0]),
                                           reason="act-table group: sqrt after exp")
                            last_exp[0] = None
                        last_sqrt = s
                    if i == 0:
                        # debug tap: D values of the first row-tile
                        nc.sync.dma_start(out=dbg_d[:, :], in_=st[:])
                    staged.append((st, i, last_sqrt))
                    if len(staged) == G or i == MT - 1:
                        flush()

    nc.compile()
    return nc


def _get_nc():
    global _nc_cache
    if _nc_cache is None:
        _nc_cache = _build()
    return _nc_cache


def _make_in_maps(inputs1, inputs2):
    inputs1 = np.asarray(inputs1, dtype=np.float32)
    inputs2 = np.asarray(inputs2, dtype=np.float32)
    in_maps = []
    for c in range(N_CORES):
        b, h = divmod(c, 2)
        a = inputs1[b, h * M:(h + 1) * M, :]
        in_maps.append({
            "aT": np.ascontiguousarray(a.T),
            "bT": np.ascontiguousarray(inputs2[b].T),
            "a": np.ascontiguousarray(a),
        })
    return in_maps


def _run_spmd(inputs1, inputs2, trace=False):
    nc = _get_nc()
    in_maps = _make_in_maps(inputs1, inputs2)
    return run_bass_kernel_spmd(nc, in_maps, core_ids=list(range(N_CORES)),
                                trace=trace)


def _assemble(results):
    out = np.empty((B_FULL, 2 * M, N_FULL), np.float32)
    for c in range(N_CORES):
        b, h = divmod(c, 2)
        out[b, h * M:(h + 1) * M, :] = results[c]["o"]
    return out


def kernel(inputs1, inputs2):
    res = _run_spmd(inputs1, inputs2, trace=False)
    return _assemble(res.results)
